# revision 16
# baseline (speedup 1.0000x reference)
"""Trainium2 Bass kernel for GQA causal sliding-window attention.

Problem (hardcoded): B=2, T=4096, C=2048, H=16 q-heads, HK=8 kv-heads,
D=128, window W=512, fp32 I/O.

Sharding: 8 cores = data-parallel over batch (2) x kv-head-groups (4).
Core c handles batch b=c//4 and kv heads [2g, 2g+2) with g=c%4 (query
heads [4g, 4g+4)).  Each core computes q/k/v projections + ve-gating +
rope + rms-norm + windowed attention for its heads, then its slice of
the output projection (row-sharded c_proj).  The host sums the 4
partial projections per batch.

All matmuls run in float32r (full-rate PE mode, ~2e-4 rel err).
"""

import sys

sys.path.insert(0, "/opt/trn_rl_repo")

import math
import threading

import numpy as np

import concourse.bacc as bacc
import concourse.tile as tile
from concourse import mybir
from concourse.bass_utils import run_bass_kernel_spmd

F32 = mybir.dt.float32
F32R = mybir.dt.float32r
AF = mybir.ActivationFunctionType
OP = mybir.AluOpType

T, C = 4096, 2048
D = 128
HL, KL = 4, 2            # local q heads, local kv heads
CL_Q, CL_KV = HL * D, KL * D   # 512, 256
W = 512
TCH, NSUB, NCH = 512, 4, 8     # t-chunk, subtiles per chunk, chunks
NCC = C // 128                 # 16 contraction chunks
QC = 256                       # attention q-chunk
SCALE = 1.0 / math.sqrt(D)
EPS = float(np.finfo(np.float32).eps)


def _build_program():
    nc = bacc.Bacc("TRN2", target_bir_lowering=False, debug=False)

    xT = nc.dram_tensor("xT", [C, T], F32R, kind="ExternalInput").ap()
    ve = nc.dram_tensor("ve", [T, CL_KV], F32, kind="ExternalInput").ap()
    cos2 = nc.dram_tensor("cos2", [T, D], F32, kind="ExternalInput").ap()
    sin2 = nc.dram_tensor("sin2", [T, D], F32, kind="ExternalInput").ap()
    wall = nc.dram_tensor("wall", [C, CL_Q + CL_KV * 2], F32R, kind="ExternalInput").ap()
    wg = nc.dram_tensor("wg", [32, KL], F32R, kind="ExternalInput").ap()
    wproj = nc.dram_tensor("wproj", [CL_Q, C], F32R, kind="ExternalInput").ap()
    onesd = nc.dram_tensor("onesd", [128, 128], F32R, kind="ExternalInput").ap()
    identd = nc.dram_tensor("identd", [128, 128], F32R, kind="ExternalInput").ap()
    out = nc.dram_tensor("out", [T, C], F32, kind="ExternalOutput").ap()

    with tile.TileContext(nc) as tc:
        _body(nc, tc, xT, ve, cos2, sin2, wall, wg, wproj, onesd, identd, out)
    nc.compile()
    return nc


def _body(nc, tc, xT, ve, cos2, sin2, wall, wg, wproj, onesd, identd, out):
    from contextlib import ExitStack

    ctx = ExitStack()
    wpool = ctx.enter_context(tc.tile_pool(name="wpool", bufs=1))
    xpool = ctx.enter_context(tc.tile_pool(name="xpool", bufs=18))
    cspool = ctx.enter_context(tc.tile_pool(name="cspool", bufs=4))
    vepool = ctx.enter_context(tc.tile_pool(name="vepool", bufs=2))
    qsbp = ctx.enter_context(tc.tile_pool(name="qsbp", bufs=2))
    ropep = ctx.enter_context(tc.tile_pool(name="ropep", bufs=2))
    vecp = ctx.enter_context(tc.tile_pool(name="vecp", bufs=8))
    qnp = ctx.enter_context(tc.tile_pool(name="qnp", bufs=5))
    knp = ctx.enter_context(tc.tile_pool(name="knp", bufs=5))
    qtp = ctx.enter_context(tc.tile_pool(name="qtp", bufs=5))
    ktp = ctx.enter_context(tc.tile_pool(name="ktp", bufs=5))
    vtp = ctx.enter_context(tc.tile_pool(name="vtp", bufs=9))
    pp = ctx.enter_context(tc.tile_pool(name="pp", bufs=2))
    recp = ctx.enter_context(tc.tile_pool(name="recp", bufs=2))
    ytp = ctx.enter_context(tc.tile_pool(name="ytp", bufs=9))
    osbp = ctx.enter_context(tc.tile_pool(name="osbp", bufs=3))

    qkvps = ctx.enter_context(tc.tile_pool(name="qkvps", bufs=1, space="PSUM"))
    sps = ctx.enter_context(tc.tile_pool(name="sps", bufs=2, space="PSUM"))
    smallps = ctx.enter_context(tc.tile_pool(name="smallps", bufs=2, space="PSUM"))

    # ---- resident weights / constants ----
    wall_t = []
    for cc in range(NCC):
        t = wpool.tile([128, CL_Q + 2 * CL_KV], F32R, name=f"wall{cc}", tag=f"wall{cc}")
        nc.sync.dma_start(t[:], wall[cc * 128:(cc + 1) * 128, :])
        wall_t.append(t)
    wproj_t = []
    for h in range(HL):
        t = wpool.tile([128, C], F32R, name=f"wproj{h}", tag=f"wproj{h}")
        nc.sync.dma_start(t[:], wproj[h * 128:(h + 1) * 128, :])
        wproj_t.append(t)
    wg_t = wpool.tile([32, KL], F32R, name="wgt", tag="wgt")
    nc.sync.dma_start(wg_t[:], wg[:])
    ones_t = wpool.tile([128, 128], F32R, name="onest", tag="onest")
    nc.sync.dma_start(ones_t[:], onesd[:])
    ident_t = wpool.tile([128, 128], F32R, name="identt", tag="identt")
    nc.sync.dma_start(ident_t[:], identd[:])
    eps_t = wpool.tile([128, 1], F32, name="epst", tag="epst")
    nc.gpsimd.memset(eps_t[:], EPS)

    kT_sb = {}   # (tch, kh) -> tile [128 d, 512 t]
    v_sb = {}    # global subtile idx -> tile [128 t, 256 d]
    yT_sb = {}   # (h, qc_global) -> tile [128 d, 256 q]

    for tch in range(NCH):
        t0 = tch * TCH
        qn_sub, kn_sub = [], []
        xts = None
        for sub in range(NSUB):
            ts = t0 + sub * 128
            si = ts // 128
            if sub % 2 == 0:
                # load xT for this half chunk (2 subtiles)
                xts = []
                for cc in range(NCC):
                    t = xpool.tile([128, 256], F32R, name=f"xt{si}_{cc}", tag="xt")
                    nc.sync.dma_start(t[:], xT[cc * 128:(cc + 1) * 128, ts:ts + 256])
                    xts.append(t)
            # ---- qkv projections ----
            qkv = qkvps.tile([128, 1024], F32, name=f"qkv{si}", tag="qkv")
            for cc in range(NCC):
                lhs = xts[cc][:, (sub % 2) * 128:(sub % 2 + 1) * 128]
                nc.tensor.matmul(qkv[:, 0:512], lhs, wall_t[cc][:, 0:512],
                                 start=(cc == 0), stop=(cc == NCC - 1))
            for cc in range(NCC):
                lhs = xts[cc][:, (sub % 2) * 128:(sub % 2 + 1) * 128]
                nc.tensor.matmul(qkv[:, 512:1024], lhs, wall_t[cc][:, 512:1024],
                                 start=(cc == 0), stop=(cc == NCC - 1))
            # evict qkv PSUM fast so the next subtile's matmuls aren't
            # serialized behind the rope DVE reads
            qsb = qsbp.tile([128, 1024], F32, name=f"qsb{si}", tag="qsb")
            nc.scalar.copy(qsb[:], qkv[:])
            # ---- ve gate: 2*sigmoid(x[:, :32] @ wg.T) ----
            gps = smallps.tile([128, 512], F32, name=f"gps{si}", tag="small")
            nc.tensor.matmul(gps[:, 0:KL], xts[0][0:32, (sub % 2) * 128:(sub % 2 + 1) * 128],
                             wg_t[:], start=True, stop=True)
            gexp = vecp.tile([128, 8], F32, name=f"gexp{si}", tag="vec")
            nc.scalar.activation(gexp[:, 0:KL], gps[:, 0:KL], AF.Exp, scale=-1.0)
            gden = vecp.tile([128, 8], F32, name=f"gden{si}", tag="vec")
            nc.vector.tensor_scalar(gden[:, 0:KL], gexp[:, 0:KL], 0.5, 0.5,
                                    OP.mult, OP.add)
            ginv = vecp.tile([128, 8], F32, name=f"ginv{si}", tag="vec")
            nc.vector.reciprocal(ginv[:, 0:KL], gden[:, 0:KL])
            # ---- v = v + gate * ve  (fused eviction, f32r) ----
            vet = vepool.tile([128, CL_KV], F32, name=f"ve{si}", tag="ve")
            nc.sync.dma_start(vet[:], ve[ts:ts + 128, :])
            vt = vtp.tile([128, CL_KV], F32R, name=f"v{si}", tag="v")
            for kh in range(KL):
                nc.vector.scalar_tensor_tensor(
                    vt[:, kh * 128:(kh + 1) * 128], vet[:, kh * 128:(kh + 1) * 128],
                    ginv[:, kh:kh + 1], qsb[:, 768 + kh * 128:768 + (kh + 1) * 128],
                    OP.mult, OP.add)
            v_sb[si] = vt

            # ---- rope + rms-norm for q (4 heads) and k (2 heads) ----
            cst = cspool.tile([128, D], F32, name=f"cs{si}", tag="cs")
            nc.sync.dma_start(cst[:], cos2[ts:ts + 128, :])
            snt = cspool.tile([128, D], F32, name=f"sn{si}", tag="cs")
            nc.sync.dma_start(snt[:], sin2[ts:ts + 128, :])

            ms = vecp.tile([128, 8], F32, name=f"ms{si}", tag="vec")
            qp = ropep.tile([128, CL_Q], F32, name=f"qp{si}", tag="rw")
            kp = ropep.tile([128, CL_KV], F32, name=f"kp{si}", tag="rwk")

            def rope_head(dst, dps, ncol, msidx):
                # dst[:, :] = q*cos2 + swap_halves(q)*sin2, from qsb
                m1 = ropep.tile([128, 128], F32, name=f"m1{si}_{msidx}", tag="m1")
                nc.vector.tensor_mul(m1[:], qsb[:, dps:dps + 128], cst[:])
                m2 = ropep.tile([128, 128], F32, name=f"m2{si}_{msidx}", tag="m2")
                nc.vector.tensor_mul(m2[:, 0:64], qsb[:, dps + 64:dps + 128],
                                     snt[:, 0:64])
                nc.vector.tensor_mul(m2[:, 64:128], qsb[:, dps:dps + 64],
                                     snt[:, 64:128])
                nc.vector.tensor_add(dst[:, ncol:ncol + 128], m1[:], m2[:])
                # sum of squares for rms-norm (ACT; m1 is dead, reuse as scratch)
                nc.scalar.activation(m1[:], dst[:, ncol:ncol + 128], AF.Square,
                                     accum_out=ms[:, msidx:msidx + 1])

            for h in range(HL):
                rope_head(qp, h * 128, h * 128, h)
            for kh in range(KL):
                rope_head(kp, 512 + kh * 128, kh * 128, HL + kh)

            lnt = vecp.tile([128, 8], F32, name=f"ln{si}", tag="vec")
            nc.scalar.activation(lnt[:, 0:6], ms[:, 0:6], AF.Ln,
                                 scale=1.0 / D, bias=eps_t[:, 0:1])
            inv = vecp.tile([128, 8], F32, name=f"inv{si}", tag="vec")
            nc.scalar.activation(inv[:, 0:6], lnt[:, 0:6], AF.Exp, scale=-0.5)

            qn = qnp.tile([128, CL_Q], F32R, name=f"qn{si}", tag="qn")
            for h in range(HL):
                nc.vector.tensor_scalar_mul(qn[:, h * 128:(h + 1) * 128],
                                            qp[:, h * 128:(h + 1) * 128],
                                            inv[:, h:h + 1])
            kn = knp.tile([128, CL_KV], F32R, name=f"kn{si}", tag="kn")
            for kh in range(KL):
                nc.vector.tensor_scalar_mul(kn[:, kh * 128:(kh + 1) * 128],
                                            kp[:, kh * 128:(kh + 1) * 128],
                                            inv[:, HL + kh:HL + kh + 1])
            qn_sub.append(qn)
            kn_sub.append(kn)

        # ---- transpose q,k to [d, t] layout ----
        qT_h = []
        for h in range(HL):
            tps = smallps.tile([128, 512], F32R, name=f"tq{tch}_{h}", tag="small")
            for sub in range(NSUB):
                nc.tensor.transpose(tps[:, sub * 128:(sub + 1) * 128],
                                    qn_sub[sub][:, h * 128:(h + 1) * 128], ident_t[:])
            qt = qtp.tile([128, TCH], F32R, name=f"qT{tch}_{h}", tag="qT")
            nc.scalar.copy(qt[:], tps[:])
            qT_h.append(qt)
        for kh in range(KL):
            tps = smallps.tile([128, 512], F32R, name=f"tk{tch}_{kh}", tag="small")
            for sub in range(NSUB):
                nc.tensor.transpose(tps[:, sub * 128:(sub + 1) * 128],
                                    kn_sub[sub][:, kh * 128:(kh + 1) * 128], ident_t[:])
            kt = ktp.tile([128, TCH], F32R, name=f"kT{tch}_{kh}", tag="kT")
            nc.scalar.copy(kt[:], tps[:])
            kT_sb[(tch, kh)] = kt

        # ---- windowed attention for this chunk's queries ----
        for qc in range(2):                     # two 256-wide q chunks
            qs = t0 + qc * QC
            jmin = max(0, (W - qs) // 128)
            js = list(range(jmin, 6))
            for h in range(HL):
                kh = h // 2
                ptiles = []
                for half in range(0, len(js), 3):
                    jgrp = js[half:half + 3]
                    sp = sps.tile([128, 768], F32, name=f"s{qs}_{h}_{half}", tag="s")
                    for idx, j in enumerate(jgrp):
                        ks = qs - W + 128 * j
                        kt = kT_sb[(ks // TCH, kh)]
                        off = ks % TCH
                        nc.tensor.matmul(sp[:, idx * 256:(idx + 1) * 256],
                                         kt[:, off:off + 128],
                                         qT_h[h][:, qc * QC:(qc + 1) * QC],
                                         start=True, stop=True)
                    p = pp.tile([128, 768], F32R, name=f"p{qs}_{h}_{half}", tag="p")
                    n = len(jgrp) * 256
                    nc.scalar.activation(p[:, 0:n], sp[:, 0:n], AF.Exp, scale=SCALE)
                    for idx, j in enumerate(jgrp):
                        sl = p[:, idx * 256:(idx + 1) * 256]
                        if j >= 4:      # lower boundary: keep iff q-k >= 0
                            nc.gpsimd.affine_select(
                                sl, sl, [[1, QC]], OP.is_ge, 0.0,
                                base=W - 128 * j, channel_multiplier=-1)
                        elif j <= 1:    # upper boundary: keep iff q-k <= W
                            nc.gpsimd.affine_select(
                                sl, sl, [[-1, QC]], OP.is_ge, 0.0,
                                base=128 * j, channel_multiplier=1)
                    ptiles.append((p, jgrp))
                yl = smallps.tile([128, 512], F32, name=f"yl{qs}_{h}", tag="small")
                nj = len(js)
                cnt = 0
                for p, jgrp in ptiles:
                    for idx, j in enumerate(jgrp):
                        ks = qs - W + 128 * j
                        vt = v_sb[ks // 128]
                        psl = p[:, idx * 256:(idx + 1) * 256]
                        nc.tensor.matmul(yl[:, 0:256],
                                         vt[:, kh * 128:(kh + 1) * 128], psl,
                                         start=(cnt == 0), stop=(cnt == nj - 1))
                        cnt += 1
                cnt = 0
                for p, jgrp in ptiles:
                    for idx, j in enumerate(jgrp):
                        psl = p[:, idx * 256:(idx + 1) * 256]
                        nc.tensor.matmul(yl[:, 256:512], ones_t[:], psl,
                                         start=(cnt == 0), stop=(cnt == nj - 1))
                        cnt += 1
                rec = recp.tile([128, 256], F32, name=f"rec{qs}_{h}", tag="rec")
                nc.vector.reciprocal(rec[:], yl[:, 256:512])
                yt = ytp.tile([128, QC], F32R, name=f"y{qs}_{h}", tag="y")
                nc.vector.tensor_mul(yt[:], yl[:, 0:256], rec[:])
                yT_sb[(h, qs)] = yt

        # ---- output projection (row-sharded c_proj partial) ----
        for sub in range(NSUB):
            ts = t0 + sub * 128
            qs = t0 + (sub // 2) * QC
            half = (sub % 2) * 128
            for oc in range(4):
                ops_ = smallps.tile([128, 512], F32, name=f"o{ts}_{oc}", tag="small")
                for h in range(HL):
                    nc.tensor.matmul(ops_[:],
                                     yT_sb[(h, qs)][:, half:half + 128],
                                     wproj_t[h][:, oc * 512:(oc + 1) * 512],
                                     start=(h == 0), stop=(h == HL - 1))
                osb = osbp.tile([128, 512], F32, name=f"ob{ts}_{oc}", tag="ob")
                nc.scalar.copy(osb[:], ops_[:])
                nc.sync.dma_start(out[ts:ts + 128, oc * 512:(oc + 1) * 512], osb[:])

    ctx.close()


_CACHE = {}
_LOCK = threading.Lock()
TRACE = False        # set True (e.g. from test.py) to capture an NTFF trace
RUN_KWARGS = {}


def _get_program():
    with _LOCK:
        if "nc" not in _CACHE:
            _CACHE["nc"] = _build_program()
        return _CACHE["nc"]


def _make_in_maps(inputs):
    return _prep_in_maps(
        inputs["x"], inputs["ve"], inputs["cos"], inputs["sin"], inputs["wq"],
        inputs["wk"], inputs["wv"], inputs["wproj"], inputs["wgate"])


def _prep_in_maps(x, ve, cos, sin, wq, wk, wv, wproj, wgate):
    x = np.asarray(x, dtype=np.float32)
    ve = np.asarray(ve, dtype=np.float32)
    cosn = np.asarray(cos, dtype=np.float32).reshape(T, D // 2)
    sinn = np.asarray(sin, dtype=np.float32).reshape(T, D // 2)
    wq = np.asarray(wq, dtype=np.float32)
    wk = np.asarray(wk, dtype=np.float32)
    wv = np.asarray(wv, dtype=np.float32)
    wproj = np.asarray(wproj, dtype=np.float32)
    wgate = np.asarray(wgate, dtype=np.float32)

    cos2 = np.ascontiguousarray(np.concatenate([cosn, cosn], axis=1))
    sin2 = np.ascontiguousarray(np.concatenate([sinn, -sinn], axis=1))
    onesd = np.ones((128, 128), dtype=np.float32)
    identd = np.eye(128, dtype=np.float32)

    xT_b = [np.ascontiguousarray(x[b].T) for b in range(2)]
    in_maps = []
    for core in range(8):
        b, g = core // 4, core % 4
        wallg = np.ascontiguousarray(np.concatenate([
            wq[g * CL_Q:(g + 1) * CL_Q].T,
            wk[g * CL_KV:(g + 1) * CL_KV].T,
            wv[g * CL_KV:(g + 1) * CL_KV].T], axis=1))
        in_maps.append({
            "xT": xT_b[b],
            "ve": np.ascontiguousarray(ve[b, :, g * CL_KV:(g + 1) * CL_KV]),
            "cos2": cos2,
            "sin2": sin2,
            "wall": wallg,
            "wg": np.ascontiguousarray(wgate[g * KL:(g + 1) * KL].T),
            "wproj": np.ascontiguousarray(wproj[:, g * CL_Q:(g + 1) * CL_Q].T),
            "onesd": onesd,
            "identd": identd,
        })
    return in_maps


def kernel(x, ve, cos, sin, wq, wk, wv, wproj, wgate, window_size):
    assert int(window_size) == W
    in_maps = _prep_in_maps(x, ve, cos, sin, wq, wk, wv, wproj, wgate)
    nc = _get_program()
    res = run_bass_kernel_spmd(nc, in_maps, core_ids=list(range(8)),
                               trace=TRACE, **RUN_KWARGS)
    _CACHE["last_results"] = res
    outs = [r["out"] for r in res.results]
    full = np.empty((2, T, C), dtype=np.float32)
    for b in range(2):
        full[b] = outs[b * 4] + outs[b * 4 + 1] + outs[b * 4 + 2] + outs[b * 4 + 3]
    return full


# revision 29
# speedup vs baseline: 73.5521x; 73.5521x over previous
"""Trainium2 Bass kernel for GQA causal sliding-window attention.

Problem (hardcoded): B=2, T=4096, C=2048, H=16 q-heads, HK=8 kv-heads,
D=128, window W=512, fp32 I/O.

Sharding: 8 cores = data-parallel over batch (2) x kv-head-groups (4).
Core c handles batch b=c//4 and kv heads [2g, 2g+2) with g=c%4 (query
heads [4g, 4g+4)).  Each core computes q/k/v projections + ve-gating +
rope + rms-norm + windowed attention for its heads, then its slice of
the output projection (row-sharded c_proj).  The host sums the 4
partial projections per batch.

All matmuls run in float32r (full-rate PE mode, ~2e-4 rel err).
"""

import sys

sys.path.insert(0, "/opt/trn_rl_repo")

import math
import threading

import numpy as np

import concourse.bacc as bacc
import concourse.tile as tile
from concourse import mybir
from concourse.bass_utils import run_bass_kernel_spmd

F32 = mybir.dt.float32
F32R = mybir.dt.float32r
AF = mybir.ActivationFunctionType
OP = mybir.AluOpType

T, C = 4096, 2048
D = 128
HL, KL = 4, 2            # local q heads, local kv heads
CL_Q, CL_KV = HL * D, KL * D   # 512, 256
W = 512
TCH, NSUB, NCH = 512, 4, 8     # t-chunk, subtiles per chunk, chunks
NCC = C // 128                 # 16 contraction chunks
QC = 256                       # attention q-chunk
SCALE = 1.0 / math.sqrt(D)
EPS = float(np.finfo(np.float32).eps)


def _pin_act_table():
    """Restrict the act-table chooser to natural_log_exp_and_others (it
    contains every ACT func this kernel uses: exp, ln, copy, identity,
    square), so the whole kernel needs exactly one table load instead of
    thrashing exp<->ln sets."""
    import concourse.hw_specs as hw_specs
    orig = hw_specs.get_activation_tables

    def pinned(arch):
        full = orig(arch)
        # keep dict size/order so act_func_set_id indices stay valid;
        # empty out every other set so the chooser can't pick them
        return {name: (fns if name == "natural_log_exp_and_others" else set())
                for name, fns in full.items()}

    bacc.get_activation_tables = pinned


_pin_act_table()


def _build_program(loop_reps=None):
    nc = bacc.Bacc("TRN2", target_bir_lowering=False, debug=False)

    xT = nc.dram_tensor("xT", [C, T], F32R, kind="ExternalInput").ap()
    ve = nc.dram_tensor("ve", [T, CL_KV], F32, kind="ExternalInput").ap()
    cs2 = nc.dram_tensor("cs2", [T, 2 * D], F32, kind="ExternalInput").ap()
    wall = nc.dram_tensor("wall", [C, CL_Q + CL_KV * 2], F32R, kind="ExternalInput").ap()
    wg = nc.dram_tensor("wg", [32, KL], F32R, kind="ExternalInput").ap()
    wproj = nc.dram_tensor("wproj", [CL_Q, C], F32R, kind="ExternalInput").ap()
    onesd = nc.dram_tensor("onesd", [128, 128], F32R, kind="ExternalInput").ap()
    identd = nc.dram_tensor("identd", [128, 128], F32R, kind="ExternalInput").ap()
    out = nc.dram_tensor("out", [T, C], F32, kind="ExternalOutput").ap()

    with tile.TileContext(nc) as tc:
        if loop_reps is None:
            _body(nc, tc, xT, ve, cs2, wall, wg, wproj, onesd, identd, out)
        else:
            with tc.For_i(0, loop_reps, 1):
                _body(nc, tc, xT, ve, cs2, wall, wg, wproj, onesd, identd, out)
    nc.compile()
    return nc


def _body(nc, tc, xT, ve, cs2, wall, wg, wproj, onesd, identd, out):
    from contextlib import ExitStack

    ctx = ExitStack()
    wpool = ctx.enter_context(tc.tile_pool(name="wpool", bufs=1))
    xpool = ctx.enter_context(tc.tile_pool(name="xpool", bufs=17))
    cspool = ctx.enter_context(tc.tile_pool(name="cspool", bufs=3))
    vepool = ctx.enter_context(tc.tile_pool(name="vepool", bufs=2))
    qsbp = ctx.enter_context(tc.tile_pool(name="qsbp", bufs=2))
    ropep = ctx.enter_context(tc.tile_pool(name="ropep", bufs=2))
    vecp = ctx.enter_context(tc.tile_pool(name="vecp", bufs=8))
    qnp = ctx.enter_context(tc.tile_pool(name="qnp", bufs=4))
    knp = ctx.enter_context(tc.tile_pool(name="knp", bufs=4))
    qtp = ctx.enter_context(tc.tile_pool(name="qtp", bufs=4))
    ktp = ctx.enter_context(tc.tile_pool(name="ktp", bufs=5))
    vtp = ctx.enter_context(tc.tile_pool(name="vtp", bufs=9))
    pp = ctx.enter_context(tc.tile_pool(name="pp", bufs=3))
    recp = ctx.enter_context(tc.tile_pool(name="recp", bufs=2))
    ytp = ctx.enter_context(tc.tile_pool(name="ytp", bufs=9))
    osbp = ctx.enter_context(tc.tile_pool(name="osbp", bufs=3))

    qkvps = ctx.enter_context(tc.tile_pool(name="qkvps", bufs=2, space="PSUM"))
    sps = ctx.enter_context(tc.tile_pool(name="sps", bufs=2, space="PSUM"))
    smallps = ctx.enter_context(tc.tile_pool(name="smallps", bufs=2, space="PSUM"))

    # ---- resident weights / constants (wall DMAs interleaved with the
    # first chunk's xT loads below so the first matmuls start early) ----
    wall_t = [None] * NCC
    wproj_t = []
    wg_t = wpool.tile([32, KL], F32R, name="wgt", tag="wgt")
    nc.sync.dma_start(wg_t[:], wg[:])
    ones_t = wpool.tile([128, 128], F32R, name="onest", tag="onest")
    nc.sync.dma_start(ones_t[:], onesd[:])
    ident_t = wpool.tile([128, 128], F32R, name="identt", tag="identt")
    nc.sync.dma_start(ident_t[:], identd[:])
    eps_t = wpool.tile([128, 1], F32, name="epst", tag="epst")
    nc.gpsimd.memset(eps_t[:], EPS)

    kT_sb = {}   # (tch, kh) -> tile [128 d, 512 t]
    v_sb = {}    # global subtile idx -> tile [128 t, 256 d]
    yT_sb = {}   # (h, qc_global) -> tile [128 d, 256 q]

    for tch in range(NCH):
        t0 = tch * TCH
        qn_sub, kn_sub = [], []
        xts = None
        half_state = {}
        for sub in range(NSUB):
            ts = t0 + sub * 128
            si = ts // 128
            sh = sub % 2
            if sh == 0:
                # load xT for this half chunk (2 subtiles); split issue
                # across SP and Pool sequencers
                xts = []
                for cc in range(NCC):
                    if wall_t[cc] is None:
                        wt = wpool.tile([128, CL_Q + 2 * CL_KV], F32R,
                                        name=f"wall{cc}", tag=f"wall{cc}")
                        nc.sync.dma_start(wt[:], wall[cc * 128:(cc + 1) * 128, :])
                        wall_t[cc] = wt
                    t = xpool.tile([128, 256], F32R, name=f"xt{si}_{cc}", tag="xt")
                    nc.sync.dma_start(t[:], xT[cc * 128:(cc + 1) * 128, ts:ts + 256])
                    xts.append(t)
                # per-half-chunk rms-norm scratch: cols (sub_in_half*6 + idx)
                ms = vecp.tile([128, 16], F32, name=f"ms{si}", tag="ms")
                half_state = {"ms": ms, "qps": [], "kps": []}
            ms = half_state["ms"]
            # ---- qkv projections ----
            qkv = qkvps.tile([128, 1024], F32, name=f"qkv{si}", tag="qkv")
            for cc in range(NCC):
                lhs = xts[cc][:, sh * 128:(sh + 1) * 128]
                nc.tensor.matmul(qkv[:, 0:512], lhs, wall_t[cc][:, 0:512],
                                 start=(cc == 0), stop=(cc == NCC - 1))
            for cc in range(NCC):
                lhs = xts[cc][:, sh * 128:(sh + 1) * 128]
                nc.tensor.matmul(qkv[:, 512:1024], lhs, wall_t[cc][:, 512:1024],
                                 start=(cc == 0), stop=(cc == NCC - 1))
            # evict qkv PSUM fast so the next subtile's matmuls aren't
            # serialized behind the rope DVE reads
            qsb = qsbp.tile([128, 1024], F32, name=f"qsb{si}", tag="qsb")
            nc.scalar.copy(qsb[:], qkv[:])
            # ---- ve gate: 2*sigmoid(x[:, :32] @ wg.T) ----
            gps = smallps.tile([128, 512], F32, name=f"gps{si}", tag="small")
            nc.tensor.matmul(gps[:, 0:KL], xts[0][0:32, sh * 128:(sh + 1) * 128],
                             wg_t[:], start=True, stop=True)
            gexp = vecp.tile([128, 8], F32, name=f"gexp{si}", tag="vec")
            nc.scalar.activation(gexp[:, 0:KL], gps[:, 0:KL], AF.Exp, scale=-1.0)
            gden = vecp.tile([128, 8], F32, name=f"gden{si}", tag="vec")
            nc.vector.tensor_scalar(gden[:, 0:KL], gexp[:, 0:KL], 0.5, 0.5,
                                    OP.mult, OP.add)
            ginv = vecp.tile([128, 8], F32, name=f"ginv{si}", tag="vec")
            nc.vector.reciprocal_approx_fast(ginv[:, 0:KL], gden[:, 0:KL])
            # ---- v = v + gate * ve  (fused eviction, f32r) ----
            vet = vepool.tile([128, CL_KV], F32, name=f"ve{si}", tag="ve")
            nc.gpsimd.dma_start(vet[:], ve[ts:ts + 128, :])
            vt = vtp.tile([128, CL_KV], F32R, name=f"v{si}", tag="v")
            for kh in range(KL):
                nc.vector.scalar_tensor_tensor(
                    vt[:, kh * 128:(kh + 1) * 128], vet[:, kh * 128:(kh + 1) * 128],
                    ginv[:, kh:kh + 1], qsb[:, 768 + kh * 128:768 + (kh + 1) * 128],
                    OP.mult, OP.add)
            v_sb[si] = vt

            # ---- rope for q (4 heads) and k (2 heads), batched over heads ----
            cst = cspool.tile([128, 2 * D], F32, name=f"cs{si}", tag="cs")
            nc.gpsimd.dma_start(cst[:], cs2[ts:ts + 128, :])

            def rope(dps, nh, dst_tag):
                # dst = qsb[:, dps:dps+nh*128]*cos2 + swap64(qsb)*sin2
                n = nh * 128
                cosb = cst[:, 0:128].rearrange("p (o d) -> p o d", o=1) \
                    .to_broadcast([128, nh, 128])
                m1 = ropep.tile([128, n], F32, name=f"m1{si}_{nh}", tag=f"m1{dst_tag}")
                nc.vector.tensor_tensor(
                    m1[:].rearrange("p (h d) -> p h d", h=nh),
                    qsb[:, dps:dps + n].rearrange("p (h d) -> p h d", h=nh),
                    cosb, OP.mult)
                m2 = ropep.tile([128, n], F32, name=f"m2{si}_{nh}", tag=f"m2{dst_tag}")
                qv = qsb[:, dps:dps + n].rearrange("p (h s d) -> p h s d", h=nh, s=2)
                o2 = m2[:].rearrange("p (h s d) -> p h s d", h=nh, s=2)
                sinb0 = cst[:, 128:192].rearrange("p (o d) -> p o d", o=1) \
                    .to_broadcast([128, nh, 64])
                sinb1 = cst[:, 192:256].rearrange("p (o d) -> p o d", o=1) \
                    .to_broadcast([128, nh, 64])
                nc.vector.tensor_tensor(o2[:, :, 0, :], qv[:, :, 1, :], sinb0, OP.mult)
                nc.vector.tensor_tensor(o2[:, :, 1, :], qv[:, :, 0, :], sinb1, OP.mult)
                dst = ropep.tile([128, n], F32, name=f"rp{si}_{nh}", tag=dst_tag)
                nc.vector.tensor_add(dst[:], m1[:], m2[:])
                # sum of squares on gpsimd (m1 dead -> scratch), reduce on DVE
                nc.gpsimd.tensor_tensor(m1[:], dst[:], dst[:], OP.mult)
                return dst, m1

            qp, sqq = rope(0, HL, "rw")
            nc.vector.tensor_reduce(
                ms[:, sh * 6:sh * 6 + HL], sqq[:].rearrange("p (h d) -> p h d", h=HL),
                mybir.AxisListType.X, OP.add)
            kp, sqk = rope(512, KL, "rwk")
            nc.vector.tensor_reduce(
                ms[:, sh * 6 + HL:sh * 6 + 6],
                sqk[:].rearrange("p (h d) -> p h d", h=KL),
                mybir.AxisListType.X, OP.add)
            half_state["qps"].append(qp)
            half_state["kps"].append(kp)

            if sh == 1:
                # ---- per-half-chunk rms-norm: inv = rsqrt(ms/D + eps) ----
                lnt = vecp.tile([128, 16], F32, name=f"ln{si}", tag="ms")
                nc.scalar.activation(lnt[:, 0:12], ms[:, 0:12], AF.Ln,
                                     scale=1.0 / D, bias=eps_t[:, 0:1])
                inv = vecp.tile([128, 16], F32, name=f"inv{si}", tag="ms")
                nc.scalar.activation(inv[:, 0:12], lnt[:, 0:12], AF.Exp, scale=-0.5)
                for s2 in range(2):
                    qp2 = half_state["qps"][s2]
                    kp2 = half_state["kps"][s2]
                    qn = qnp.tile([128, CL_Q], F32R, name=f"qn{si}_{s2}", tag="qn")
                    for h in range(HL):
                        nc.gpsimd.tensor_scalar_mul(
                            qn[:, h * 128:(h + 1) * 128],
                            qp2[:, h * 128:(h + 1) * 128],
                            inv[:, s2 * 6 + h:s2 * 6 + h + 1])
                    kn = knp.tile([128, CL_KV], F32R, name=f"kn{si}_{s2}", tag="kn")
                    for kh in range(KL):
                        nc.gpsimd.tensor_scalar_mul(
                            kn[:, kh * 128:(kh + 1) * 128],
                            kp2[:, kh * 128:(kh + 1) * 128],
                            inv[:, s2 * 6 + HL + kh:s2 * 6 + HL + kh + 1])
                    qn_sub.append(qn)
                    kn_sub.append(kn)

        if tch == 0:
            # load wproj late so chunk-0 xT DMAs get the DMA engines first
            for h in range(HL):
                t = wpool.tile([128, C], F32R, name=f"wproj{h}", tag=f"wproj{h}")
                nc.sync.dma_start(t[:], wproj[h * 128:(h + 1) * 128, :])
                wproj_t.append(t)

        # ---- transpose q,k to [d, t] layout ----
        qT_h = []
        for h in range(HL):
            tps = smallps.tile([128, 512], F32R, name=f"tq{tch}_{h}", tag="small")
            for sub in range(NSUB):
                nc.tensor.transpose(tps[:, sub * 128:(sub + 1) * 128],
                                    qn_sub[sub][:, h * 128:(h + 1) * 128], ident_t[:])
            qt = qtp.tile([128, TCH], F32R, name=f"qT{tch}_{h}", tag="qT")
            nc.scalar.copy(qt[:], tps[:])
            qT_h.append(qt)
        for kh in range(KL):
            tps = smallps.tile([128, 512], F32R, name=f"tk{tch}_{kh}", tag="small")
            for sub in range(NSUB):
                nc.tensor.transpose(tps[:, sub * 128:(sub + 1) * 128],
                                    kn_sub[sub][:, kh * 128:(kh + 1) * 128], ident_t[:])
            kt = ktp.tile([128, TCH], F32R, name=f"kT{tch}_{kh}", tag="kT")
            nc.scalar.copy(kt[:], tps[:])
            kT_sb[(tch, kh)] = kt

        # ---- windowed attention for this chunk's queries ----
        for qc in range(2):                     # two 256-wide q chunks
            qs = t0 + qc * QC
            jmin = max(0, (W - qs) // 128)
            js = list(range(jmin, 6))
            for h in range(HL):
                kh = h // 2
                pslices = {}    # j -> p slice AP
                for gi in range(0, len(js), 2):
                    jgrp = js[gi:gi + 2]
                    sp = sps.tile([128, 512], F32, name=f"s{qs}_{h}_{gi}", tag="s")
                    for idx, j in enumerate(jgrp):
                        ks = qs - W + 128 * j
                        kt = kT_sb[(ks // TCH, kh)]
                        off = ks % TCH
                        nc.tensor.matmul(sp[:, idx * 256:(idx + 1) * 256],
                                         kt[:, off:off + 128],
                                         qT_h[h][:, qc * QC:(qc + 1) * QC],
                                         start=True, stop=True)
                    p = pp.tile([128, 512], F32R, name=f"p{qs}_{h}_{gi}", tag="p")
                    n = len(jgrp) * 256
                    nc.scalar.activation(p[:, 0:n], sp[:, 0:n], AF.Exp, scale=SCALE)
                    for idx, j in enumerate(jgrp):
                        sl = p[:, idx * 256:(idx + 1) * 256]
                        if j >= 4:      # lower boundary: keep iff q-k >= 0
                            nc.gpsimd.affine_select(
                                sl, sl, [[1, QC]], OP.is_ge, 0.0,
                                base=W - 128 * j, channel_multiplier=-1)
                        elif j <= 1:    # upper boundary: keep iff q-k <= W
                            nc.gpsimd.affine_select(
                                sl, sl, [[-1, QC]], OP.is_ge, 0.0,
                                base=128 * j, channel_multiplier=1)
                        pslices[j] = sl
                yl = smallps.tile([128, 512], F32, name=f"yl{qs}_{h}", tag="small")
                nj = len(js)
                # accumulate unmasked chunks first so the gpsimd mask
                # selects on masked ones can complete in the shadow
                jorder = [j for j in js if 2 <= j <= 3] + \
                         [j for j in js if j < 2 or j > 3]
                for cnt, j in enumerate(jorder):
                    ks = qs - W + 128 * j
                    vt = v_sb[ks // 128]
                    nc.tensor.matmul(yl[:, 0:256],
                                     vt[:, kh * 128:(kh + 1) * 128], pslices[j],
                                     start=(cnt == 0), stop=(cnt == nj - 1))
                for cnt, j in enumerate(jorder):
                    nc.tensor.matmul(yl[:, 256:512], ones_t[:], pslices[j],
                                     start=(cnt == 0), stop=(cnt == nj - 1))
                rec = recp.tile([128, 256], F32, name=f"rec{qs}_{h}", tag="rec")
                nc.vector.reciprocal_approx_fast(rec[:], yl[:, 256:512])
                yt = ytp.tile([128, QC], F32R, name=f"y{qs}_{h}", tag="y")
                nc.vector.tensor_mul(yt[:], yl[:, 0:256], rec[:])
                yT_sb[(h, qs)] = yt

        # ---- output projection (row-sharded c_proj partial) ----
        for sub in range(NSUB):
            ts = t0 + sub * 128
            qs = t0 + (sub // 2) * QC
            half = (sub % 2) * 128
            for oc in range(4):
                ops_ = smallps.tile([128, 512], F32, name=f"o{ts}_{oc}", tag="small")
                for h in range(HL):
                    nc.tensor.matmul(ops_[:],
                                     yT_sb[(h, qs)][:, half:half + 128],
                                     wproj_t[h][:, oc * 512:(oc + 1) * 512],
                                     start=(h == 0), stop=(h == HL - 1))
                osb = osbp.tile([128, 512], F32, name=f"ob{ts}_{oc}", tag="ob")
                nc.scalar.copy(osb[:], ops_[:])
                nc.scalar.dma_start(out[ts:ts + 128, oc * 512:(oc + 1) * 512], osb[:])

    ctx.close()


_CACHE = {}
_LOCK = threading.Lock()
TRACE = False        # set True (e.g. from test.py) to capture an NTFF trace
RUN_KWARGS = {}


def _get_program():
    with _LOCK:
        if "nc" not in _CACHE:
            _CACHE["nc"] = _build_program()
        return _CACHE["nc"]


def _make_in_maps(inputs):
    return _prep_in_maps(
        inputs["x"], inputs["ve"], inputs["cos"], inputs["sin"], inputs["wq"],
        inputs["wk"], inputs["wv"], inputs["wproj"], inputs["wgate"])


def _prep_in_maps(x, ve, cos, sin, wq, wk, wv, wproj, wgate):
    x = np.asarray(x, dtype=np.float32)
    ve = np.asarray(ve, dtype=np.float32)
    cosn = np.asarray(cos, dtype=np.float32).reshape(T, D // 2)
    sinn = np.asarray(sin, dtype=np.float32).reshape(T, D // 2)
    wq = np.asarray(wq, dtype=np.float32)
    wk = np.asarray(wk, dtype=np.float32)
    wv = np.asarray(wv, dtype=np.float32)
    wproj = np.asarray(wproj, dtype=np.float32)
    wgate = np.asarray(wgate, dtype=np.float32)

    cs2 = np.ascontiguousarray(np.concatenate([cosn, cosn, sinn, -sinn], axis=1))
    onesd = np.ones((128, 128), dtype=np.float32)
    identd = np.eye(128, dtype=np.float32)

    xT_b = [np.ascontiguousarray(x[b].T) for b in range(2)]
    in_maps = []
    for core in range(8):
        b, g = core // 4, core % 4
        wallg = np.ascontiguousarray(np.concatenate([
            wq[g * CL_Q:(g + 1) * CL_Q].T,
            wk[g * CL_KV:(g + 1) * CL_KV].T,
            wv[g * CL_KV:(g + 1) * CL_KV].T], axis=1))
        in_maps.append({
            "xT": xT_b[b],
            "ve": np.ascontiguousarray(ve[b, :, g * CL_KV:(g + 1) * CL_KV]),
            "cs2": cs2,
            "wall": wallg,
            "wg": np.ascontiguousarray(wgate[g * KL:(g + 1) * KL].T),
            "wproj": np.ascontiguousarray(wproj[:, g * CL_Q:(g + 1) * CL_Q].T),
            "onesd": onesd,
            "identd": identd,
        })
    return in_maps


def kernel(x, ve, cos, sin, wq, wk, wv, wproj, wgate, window_size):
    assert int(window_size) == W
    in_maps = _prep_in_maps(x, ve, cos, sin, wq, wk, wv, wproj, wgate)
    nc = _get_program()
    res = run_bass_kernel_spmd(nc, in_maps, core_ids=list(range(8)),
                               trace=TRACE, **RUN_KWARGS)
    _CACHE["last_results"] = res
    outs = [r["out"] for r in res.results]
    full = np.empty((2, T, C), dtype=np.float32)
    for b in range(2):
        full[b] = outs[b * 4] + outs[b * 4 + 1] + outs[b * 4 + 2] + outs[b * 4 + 3]
    return full


# revision 30
# speedup vs baseline: 113.2835x; 1.5402x over previous
"""Trainium2 Bass kernel for GQA causal sliding-window attention.

Problem (hardcoded): B=2, T=4096, C=2048, H=16 q-heads, HK=8 kv-heads,
D=128, window W=512, fp32 I/O.

Sharding: 8 cores = data-parallel over batch (2) x kv-head-groups (4).
Core c handles batch b=c//4 and kv heads [2g, 2g+2) with g=c%4 (query
heads [4g, 4g+4)).  Each core computes q/k/v projections + ve-gating +
rope + rms-norm + windowed attention for its heads, then its slice of
the output projection (row-sharded c_proj).  The host sums the 4
partial projections per batch.

All matmuls run in float32r (full-rate PE mode, ~2e-4 rel err).
"""

import sys

sys.path.insert(0, "/opt/trn_rl_repo")

import math
import threading

import numpy as np

import concourse.bacc as bacc
import concourse.tile as tile
from concourse import mybir
from concourse.bass_utils import run_bass_kernel_spmd

F32 = mybir.dt.float32
F32R = mybir.dt.float32r
AF = mybir.ActivationFunctionType
OP = mybir.AluOpType

T, C = 4096, 2048
D = 128
HL, KL = 4, 2            # local q heads, local kv heads
CL_Q, CL_KV = HL * D, KL * D   # 512, 256
W = 512
TCH, NSUB, NCH = 512, 4, 8     # t-chunk, subtiles per chunk, chunks
NCC = C // 128                 # 16 contraction chunks
QC = 256                       # attention q-chunk
SCALE = 1.0 / math.sqrt(D)
EPS = float(np.finfo(np.float32).eps)


def _pin_act_table():
    """Restrict the act-table chooser to natural_log_exp_and_others (it
    contains every ACT func this kernel uses: exp, ln, copy, identity,
    square), so the whole kernel needs exactly one table load instead of
    thrashing exp<->ln sets."""
    import concourse.hw_specs as hw_specs
    orig = hw_specs.get_activation_tables

    def pinned(arch):
        full = orig(arch)
        # keep dict size/order so act_func_set_id indices stay valid;
        # empty out every other set so the chooser can't pick them
        return {name: (fns if name == "natural_log_exp_and_others" else set())
                for name, fns in full.items()}

    bacc.get_activation_tables = pinned


_pin_act_table()


def _build_program(loop_reps=None):
    nc = bacc.Bacc("TRN2", target_bir_lowering=False, debug=False)

    xT = nc.dram_tensor("xT", [C, T], F32R, kind="ExternalInput").ap()
    ve = nc.dram_tensor("ve", [T, CL_KV], F32, kind="ExternalInput").ap()
    cs2 = nc.dram_tensor("cs2", [T, 2 * D], F32, kind="ExternalInput").ap()
    wall = nc.dram_tensor("wall", [C, CL_Q + CL_KV * 2], F32R, kind="ExternalInput").ap()
    wg = nc.dram_tensor("wg", [32, KL], F32R, kind="ExternalInput").ap()
    wproj = nc.dram_tensor("wproj", [CL_Q, C], F32R, kind="ExternalInput").ap()
    onesd = nc.dram_tensor("onesd", [128, 128], F32R, kind="ExternalInput").ap()
    identd = nc.dram_tensor("identd", [128, 128], F32R, kind="ExternalInput").ap()
    out = nc.dram_tensor("out", [T, C], F32, kind="ExternalOutput").ap()

    with tile.TileContext(nc) as tc:
        if loop_reps is None:
            _body(nc, tc, xT, ve, cs2, wall, wg, wproj, onesd, identd, out)
        else:
            with tc.For_i(0, loop_reps, 1):
                _body(nc, tc, xT, ve, cs2, wall, wg, wproj, onesd, identd, out)
    nc.compile()
    return nc


def _body(nc, tc, xT, ve, cs2, wall, wg, wproj, onesd, identd, out):
    from contextlib import ExitStack

    ctx = ExitStack()
    wpool = ctx.enter_context(tc.tile_pool(name="wpool", bufs=1))
    xpool = ctx.enter_context(tc.tile_pool(name="xpool", bufs=17))
    cspool = ctx.enter_context(tc.tile_pool(name="cspool", bufs=3))
    vepool = ctx.enter_context(tc.tile_pool(name="vepool", bufs=2))
    qsbp = ctx.enter_context(tc.tile_pool(name="qsbp", bufs=2))
    ropep = ctx.enter_context(tc.tile_pool(name="ropep", bufs=2))
    vecp = ctx.enter_context(tc.tile_pool(name="vecp", bufs=8))
    qnp = ctx.enter_context(tc.tile_pool(name="qnp", bufs=4))
    knp = ctx.enter_context(tc.tile_pool(name="knp", bufs=4))
    qtp = ctx.enter_context(tc.tile_pool(name="qtp", bufs=4))
    ktp = ctx.enter_context(tc.tile_pool(name="ktp", bufs=5))
    vtp = ctx.enter_context(tc.tile_pool(name="vtp", bufs=9))
    pp = ctx.enter_context(tc.tile_pool(name="pp", bufs=3))
    recp = ctx.enter_context(tc.tile_pool(name="recp", bufs=2))
    ytp = ctx.enter_context(tc.tile_pool(name="ytp", bufs=9))
    osbp = ctx.enter_context(tc.tile_pool(name="osbp", bufs=3))

    qkvps = ctx.enter_context(tc.tile_pool(name="qkvps", bufs=2, space="PSUM"))
    sps = ctx.enter_context(tc.tile_pool(name="sps", bufs=2, space="PSUM"))
    smallps = ctx.enter_context(tc.tile_pool(name="smallps", bufs=2, space="PSUM"))

    # ---- resident weights / constants (wall DMAs interleaved with the
    # first chunk's xT loads below so the first matmuls start early) ----
    wall_t = [None] * NCC
    wproj_t = []
    wg_t = wpool.tile([32, KL], F32R, name="wgt", tag="wgt")
    nc.sync.dma_start(wg_t[:], wg[:])
    ones_t = wpool.tile([128, 128], F32R, name="onest", tag="onest")
    nc.sync.dma_start(ones_t[:], onesd[:])
    ident_t = wpool.tile([128, 128], F32R, name="identt", tag="identt")
    nc.sync.dma_start(ident_t[:], identd[:])
    eps_t = wpool.tile([128, 1], F32, name="epst", tag="epst")
    nc.gpsimd.memset(eps_t[:], EPS)

    kT_sb = {}   # (tch, kh) -> tile [128 d, 512 t]
    v_sb = {}    # global subtile idx -> tile [128 t, 256 d]
    yT_sb = {}   # (h, qc_global) -> tile [128 d, 256 q]

    for tch in range(NCH):
        t0 = tch * TCH
        qn_sub, kn_sub = [], []
        xts = None
        half_state = {}
        for sub in range(NSUB):
            ts = t0 + sub * 128
            si = ts // 128
            sh = sub % 2
            if sh == 0:
                # load xT for this half chunk (2 subtiles); split issue
                # across SP and Pool sequencers
                xts = []
                for cc in range(NCC):
                    if wall_t[cc] is None:
                        wt = wpool.tile([128, CL_Q + 2 * CL_KV], F32R,
                                        name=f"wall{cc}", tag=f"wall{cc}")
                        nc.sync.dma_start(wt[:], wall[cc * 128:(cc + 1) * 128, :])
                        wall_t[cc] = wt
                    t = xpool.tile([128, 256], F32R, name=f"xt{si}_{cc}", tag="xt")
                    nc.sync.dma_start(t[:], xT[cc * 128:(cc + 1) * 128, ts:ts + 256])
                    xts.append(t)
                # per-half-chunk rms-norm scratch: cols (sub_in_half*6 + idx)
                ms = vecp.tile([128, 16], F32, name=f"ms{si}", tag="ms")
                half_state = {"ms": ms, "qps": [], "kps": []}
            ms = half_state["ms"]
            # ---- qkv projections ----
            qkv = qkvps.tile([128, 1024], F32, name=f"qkv{si}", tag="qkv")
            for cc in range(NCC):
                lhs = xts[cc][:, sh * 128:(sh + 1) * 128]
                nc.tensor.matmul(qkv[:, 0:512], lhs, wall_t[cc][:, 0:512],
                                 start=(cc == 0), stop=(cc == NCC - 1))
            for cc in range(NCC):
                lhs = xts[cc][:, sh * 128:(sh + 1) * 128]
                nc.tensor.matmul(qkv[:, 512:1024], lhs, wall_t[cc][:, 512:1024],
                                 start=(cc == 0), stop=(cc == NCC - 1))
            # evict qkv PSUM fast so the next subtile's matmuls aren't
            # serialized behind the rope DVE reads
            qsb = qsbp.tile([128, 1024], F32, name=f"qsb{si}", tag="qsb")
            nc.scalar.copy(qsb[:], qkv[:])
            # ---- ve gate: 2*sigmoid(x[:, :32] @ wg.T) ----
            gps = smallps.tile([128, 512], F32, name=f"gps{si}", tag="small")
            nc.tensor.matmul(gps[:, 0:KL], xts[0][0:32, sh * 128:(sh + 1) * 128],
                             wg_t[:], start=True, stop=True)
            gexp = vecp.tile([128, 8], F32, name=f"gexp{si}", tag="vec")
            nc.scalar.activation(gexp[:, 0:KL], gps[:, 0:KL], AF.Exp, scale=-1.0)
            gden = vecp.tile([128, 8], F32, name=f"gden{si}", tag="vec")
            nc.vector.tensor_scalar(gden[:, 0:KL], gexp[:, 0:KL], 0.5, 0.5,
                                    OP.mult, OP.add)
            ginv = vecp.tile([128, 8], F32, name=f"ginv{si}", tag="vec")
            nc.vector.reciprocal_approx_fast(ginv[:, 0:KL], gden[:, 0:KL])
            # ---- v = v + gate * ve  (fused eviction, f32r) ----
            vet = vepool.tile([128, CL_KV], F32, name=f"ve{si}", tag="ve")
            nc.gpsimd.dma_start(vet[:], ve[ts:ts + 128, :])
            vt = vtp.tile([128, CL_KV], F32R, name=f"v{si}", tag="v")
            for kh in range(KL):
                nc.vector.scalar_tensor_tensor(
                    vt[:, kh * 128:(kh + 1) * 128], vet[:, kh * 128:(kh + 1) * 128],
                    ginv[:, kh:kh + 1], qsb[:, 768 + kh * 128:768 + (kh + 1) * 128],
                    OP.mult, OP.add)
            v_sb[si] = vt

            # ---- rope for q (4 heads) and k (2 heads), batched over heads ----
            cst = cspool.tile([128, 2 * D], F32, name=f"cs{si}", tag="cs")
            nc.gpsimd.dma_start(cst[:], cs2[ts:ts + 128, :])

            def rope(dps, nh, dst_tag):
                # dst = qsb[:, dps:dps+nh*128]*cos2 + swap64(qsb)*sin2
                n = nh * 128
                cosb = cst[:, 0:128].rearrange("p (o d) -> p o d", o=1) \
                    .to_broadcast([128, nh, 128])
                m1 = ropep.tile([128, n], F32, name=f"m1{si}_{nh}", tag=f"m1{dst_tag}")
                nc.vector.tensor_tensor(
                    m1[:].rearrange("p (h d) -> p h d", h=nh),
                    qsb[:, dps:dps + n].rearrange("p (h d) -> p h d", h=nh),
                    cosb, OP.mult)
                m2 = ropep.tile([128, n], F32, name=f"m2{si}_{nh}", tag=f"m2{dst_tag}")
                qv = qsb[:, dps:dps + n].rearrange("p (h s d) -> p h s d", h=nh, s=2)
                o2 = m2[:].rearrange("p (h s d) -> p h s d", h=nh, s=2)
                sinb0 = cst[:, 128:192].rearrange("p (o d) -> p o d", o=1) \
                    .to_broadcast([128, nh, 64])
                sinb1 = cst[:, 192:256].rearrange("p (o d) -> p o d", o=1) \
                    .to_broadcast([128, nh, 64])
                nc.vector.tensor_tensor(o2[:, :, 0, :], qv[:, :, 1, :], sinb0, OP.mult)
                nc.vector.tensor_tensor(o2[:, :, 1, :], qv[:, :, 0, :], sinb1, OP.mult)
                dst = ropep.tile([128, n], F32, name=f"rp{si}_{nh}", tag=dst_tag)
                nc.vector.tensor_add(dst[:], m1[:], m2[:])
                # sum of squares on gpsimd (m1 dead -> scratch), reduce on DVE
                nc.vector.tensor_tensor(m1[:], dst[:], dst[:], OP.mult)
                return dst, m1

            qp, sqq = rope(0, HL, "rw")
            nc.vector.tensor_reduce(
                ms[:, sh * 6:sh * 6 + HL], sqq[:].rearrange("p (h d) -> p h d", h=HL),
                mybir.AxisListType.X, OP.add)
            kp, sqk = rope(512, KL, "rwk")
            nc.vector.tensor_reduce(
                ms[:, sh * 6 + HL:sh * 6 + 6],
                sqk[:].rearrange("p (h d) -> p h d", h=KL),
                mybir.AxisListType.X, OP.add)
            half_state["qps"].append(qp)
            half_state["kps"].append(kp)

            if sh == 1:
                # ---- per-half-chunk rms-norm: inv = rsqrt(ms/D + eps) ----
                lnt = vecp.tile([128, 16], F32, name=f"ln{si}", tag="ms")
                nc.scalar.activation(lnt[:, 0:12], ms[:, 0:12], AF.Ln,
                                     scale=1.0 / D, bias=eps_t[:, 0:1])
                inv = vecp.tile([128, 16], F32, name=f"inv{si}", tag="ms")
                nc.scalar.activation(inv[:, 0:12], lnt[:, 0:12], AF.Exp, scale=-0.5)
                for s2 in range(2):
                    qp2 = half_state["qps"][s2]
                    kp2 = half_state["kps"][s2]
                    qn = qnp.tile([128, CL_Q], F32R, name=f"qn{si}_{s2}", tag="qn")
                    for h in range(HL):
                        nc.vector.tensor_scalar_mul(
                            qn[:, h * 128:(h + 1) * 128],
                            qp2[:, h * 128:(h + 1) * 128],
                            inv[:, s2 * 6 + h:s2 * 6 + h + 1])
                    kn = knp.tile([128, CL_KV], F32R, name=f"kn{si}_{s2}", tag="kn")
                    for kh in range(KL):
                        nc.vector.tensor_scalar_mul(
                            kn[:, kh * 128:(kh + 1) * 128],
                            kp2[:, kh * 128:(kh + 1) * 128],
                            inv[:, s2 * 6 + HL + kh:s2 * 6 + HL + kh + 1])
                    qn_sub.append(qn)
                    kn_sub.append(kn)

        if tch == 0:
            # load wproj late so chunk-0 xT DMAs get the DMA engines first
            for h in range(HL):
                t = wpool.tile([128, C], F32R, name=f"wproj{h}", tag=f"wproj{h}")
                nc.sync.dma_start(t[:], wproj[h * 128:(h + 1) * 128, :])
                wproj_t.append(t)

        # ---- transpose q,k to [d, t] layout ----
        qT_h = []
        for h in range(HL):
            tps = smallps.tile([128, 512], F32R, name=f"tq{tch}_{h}", tag="small")
            for sub in range(NSUB):
                nc.tensor.transpose(tps[:, sub * 128:(sub + 1) * 128],
                                    qn_sub[sub][:, h * 128:(h + 1) * 128], ident_t[:])
            qt = qtp.tile([128, TCH], F32R, name=f"qT{tch}_{h}", tag="qT")
            nc.scalar.copy(qt[:], tps[:])
            qT_h.append(qt)
        for kh in range(KL):
            tps = smallps.tile([128, 512], F32R, name=f"tk{tch}_{kh}", tag="small")
            for sub in range(NSUB):
                nc.tensor.transpose(tps[:, sub * 128:(sub + 1) * 128],
                                    kn_sub[sub][:, kh * 128:(kh + 1) * 128], ident_t[:])
            kt = ktp.tile([128, TCH], F32R, name=f"kT{tch}_{kh}", tag="kT")
            nc.scalar.copy(kt[:], tps[:])
            kT_sb[(tch, kh)] = kt

        # ---- windowed attention for this chunk's queries ----
        for qc in range(2):                     # two 256-wide q chunks
            qs = t0 + qc * QC
            jmin = max(0, (W - qs) // 128)
            js = list(range(jmin, 6))
            for h in range(HL):
                kh = h // 2
                pslices = {}    # j -> p slice AP
                for gi in range(0, len(js), 2):
                    jgrp = js[gi:gi + 2]
                    sp = sps.tile([128, 512], F32, name=f"s{qs}_{h}_{gi}", tag="s")
                    for idx, j in enumerate(jgrp):
                        ks = qs - W + 128 * j
                        kt = kT_sb[(ks // TCH, kh)]
                        off = ks % TCH
                        nc.tensor.matmul(sp[:, idx * 256:(idx + 1) * 256],
                                         kt[:, off:off + 128],
                                         qT_h[h][:, qc * QC:(qc + 1) * QC],
                                         start=True, stop=True)
                    p = pp.tile([128, 512], F32R, name=f"p{qs}_{h}_{gi}", tag="p")
                    n = len(jgrp) * 256
                    nc.scalar.activation(p[:, 0:n], sp[:, 0:n], AF.Exp, scale=SCALE)
                    for idx, j in enumerate(jgrp):
                        sl = p[:, idx * 256:(idx + 1) * 256]
                        if j >= 4:      # lower boundary: keep iff q-k >= 0
                            nc.gpsimd.affine_select(
                                sl, sl, [[1, QC]], OP.is_ge, 0.0,
                                base=W - 128 * j, channel_multiplier=-1)
                        elif j <= 1:    # upper boundary: keep iff q-k <= W
                            nc.gpsimd.affine_select(
                                sl, sl, [[-1, QC]], OP.is_ge, 0.0,
                                base=128 * j, channel_multiplier=1)
                        pslices[j] = sl
                yl = smallps.tile([128, 512], F32, name=f"yl{qs}_{h}", tag="small")
                nj = len(js)
                # accumulate unmasked chunks first so the gpsimd mask
                # selects on masked ones can complete in the shadow
                jorder = [j for j in js if 2 <= j <= 3] + \
                         [j for j in js if j < 2 or j > 3]
                for cnt, j in enumerate(jorder):
                    ks = qs - W + 128 * j
                    vt = v_sb[ks // 128]
                    nc.tensor.matmul(yl[:, 0:256],
                                     vt[:, kh * 128:(kh + 1) * 128], pslices[j],
                                     start=(cnt == 0), stop=(cnt == nj - 1))
                for cnt, j in enumerate(jorder):
                    nc.tensor.matmul(yl[:, 256:512], ones_t[:], pslices[j],
                                     start=(cnt == 0), stop=(cnt == nj - 1))
                rec = recp.tile([128, 256], F32, name=f"rec{qs}_{h}", tag="rec")
                nc.vector.reciprocal_approx_fast(rec[:], yl[:, 256:512])
                yt = ytp.tile([128, QC], F32R, name=f"y{qs}_{h}", tag="y")
                nc.vector.tensor_mul(yt[:], yl[:, 0:256], rec[:])
                yT_sb[(h, qs)] = yt

        # ---- output projection (row-sharded c_proj partial) ----
        for sub in range(NSUB):
            ts = t0 + sub * 128
            qs = t0 + (sub // 2) * QC
            half = (sub % 2) * 128
            for oc in range(4):
                ops_ = smallps.tile([128, 512], F32, name=f"o{ts}_{oc}", tag="small")
                for h in range(HL):
                    nc.tensor.matmul(ops_[:],
                                     yT_sb[(h, qs)][:, half:half + 128],
                                     wproj_t[h][:, oc * 512:(oc + 1) * 512],
                                     start=(h == 0), stop=(h == HL - 1))
                osb = osbp.tile([128, 512], F32, name=f"ob{ts}_{oc}", tag="ob")
                nc.scalar.copy(osb[:], ops_[:])
                nc.scalar.dma_start(out[ts:ts + 128, oc * 512:(oc + 1) * 512], osb[:])

    ctx.close()


_CACHE = {}
_LOCK = threading.Lock()
TRACE = False        # set True (e.g. from test.py) to capture an NTFF trace
RUN_KWARGS = {}


def _get_program():
    with _LOCK:
        if "nc" not in _CACHE:
            _CACHE["nc"] = _build_program()
        return _CACHE["nc"]


def _make_in_maps(inputs):
    return _prep_in_maps(
        inputs["x"], inputs["ve"], inputs["cos"], inputs["sin"], inputs["wq"],
        inputs["wk"], inputs["wv"], inputs["wproj"], inputs["wgate"])


def _prep_in_maps(x, ve, cos, sin, wq, wk, wv, wproj, wgate):
    x = np.asarray(x, dtype=np.float32)
    ve = np.asarray(ve, dtype=np.float32)
    cosn = np.asarray(cos, dtype=np.float32).reshape(T, D // 2)
    sinn = np.asarray(sin, dtype=np.float32).reshape(T, D // 2)
    wq = np.asarray(wq, dtype=np.float32)
    wk = np.asarray(wk, dtype=np.float32)
    wv = np.asarray(wv, dtype=np.float32)
    wproj = np.asarray(wproj, dtype=np.float32)
    wgate = np.asarray(wgate, dtype=np.float32)

    cs2 = np.ascontiguousarray(np.concatenate([cosn, cosn, sinn, -sinn], axis=1))
    onesd = np.ones((128, 128), dtype=np.float32)
    identd = np.eye(128, dtype=np.float32)

    xT_b = [np.ascontiguousarray(x[b].T) for b in range(2)]
    in_maps = []
    for core in range(8):
        b, g = core // 4, core % 4
        wallg = np.ascontiguousarray(np.concatenate([
            wq[g * CL_Q:(g + 1) * CL_Q].T,
            wk[g * CL_KV:(g + 1) * CL_KV].T,
            wv[g * CL_KV:(g + 1) * CL_KV].T], axis=1))
        in_maps.append({
            "xT": xT_b[b],
            "ve": np.ascontiguousarray(ve[b, :, g * CL_KV:(g + 1) * CL_KV]),
            "cs2": cs2,
            "wall": wallg,
            "wg": np.ascontiguousarray(wgate[g * KL:(g + 1) * KL].T),
            "wproj": np.ascontiguousarray(wproj[:, g * CL_Q:(g + 1) * CL_Q].T),
            "onesd": onesd,
            "identd": identd,
        })
    return in_maps


def kernel(x, ve, cos, sin, wq, wk, wv, wproj, wgate, window_size):
    assert int(window_size) == W
    in_maps = _prep_in_maps(x, ve, cos, sin, wq, wk, wv, wproj, wgate)
    nc = _get_program()
    res = run_bass_kernel_spmd(nc, in_maps, core_ids=list(range(8)),
                               trace=TRACE, **RUN_KWARGS)
    _CACHE["last_results"] = res
    outs = [r["out"] for r in res.results]
    full = np.empty((2, T, C), dtype=np.float32)
    for b in range(2):
        full[b] = outs[b * 4] + outs[b * 4 + 1] + outs[b * 4 + 2] + outs[b * 4 + 3]
    return full
